# revision 39
# baseline (speedup 1.0000x reference)
"""Correspondence Soft-NMS on 8 Trainium2 NeuronCores (Bass/Tile), v4.

Math: penalty_i = sum_j [s_j > s_i] * exp(-(d2src_ij + d2tgt_ij)/delta^2)
      out_i    = s_i * exp(-penalty_i / sigma)

Design (v4 = rescheduled v3):
  * Host sorts by score desc -> suppressors are the positional prefix (ties
    fixed by a host-side correction).
  * Pairwise dots are PRE-SCALED into the Schraudolph integer domain:
    psum x = A_SC*(sq_j - 2 P_i.P_j), A_SC = -100*log2(e)*2^23. With
    dp_i = A_SC*sq_i + (127*2^23 - C), rne(max(x+dp,0)) bitcast to f32
    IS ~exp(z) (exact 0 for far pairs).
  * bf16 limbs, 6 limb-pair groups (00,11,02,10,01,20) -> K=42, duplicated
    at partitions 0/64 so consecutive matmuls alternate PE row-groups.
  * 36 windows of 1024 cols (2 matmuls each) stream through a 4-window
    PSUM ring. Window order is hand-scheduled so the two consumer streams
    (ACT on scalar, Schraudolph 2-pass on vector) interleave ~2:1 and all
    ACT pair-spans are ring-contiguous (no phase wrap).
  * ACT stream: exp(SC_ACT*x + bias_i) + fused row-sum, 2048-col spans.
  * DVE stream: pass1 i32 = rne(max(x+dp,0)) per window into a dedicated
    per-slot SBUF region (boundary windows via scalar_tensor_tensor with a
    2^30 mask -> bitcast 2.0 per excluded element, fixed by the final exp
    bias); pass2 sums the bitcast region once per slot with accum_out.
  * Startup: warmup exp (table load) issued first on scalar; input DMAs
    split across sync/scalar queues smallest-first so matmul 0 starts as
    early as possible.
"""

import sys
import types

import numpy as np
import ml_dtypes


def _ensure_axon_hooks():
    try:
        import antenv.axon_hooks  # noqa: F401
        return
    except ImportError:
        pass
    try:
        import antenv
    except ImportError:
        return
    mod = types.ModuleType("antenv.axon_hooks")
    mod._hook = None

    def set_axon_ntff_profile_hook(h):
        mod._hook = h

    def get_axon_ntff_profile_hook():
        return mod._hook

    mod.set_axon_ntff_profile_hook = set_axon_ntff_profile_hook
    mod.get_axon_ntff_profile_hook = get_axon_ntff_profile_hook
    sys.modules["antenv.axon_hooks"] = mod
    antenv.axon_hooks = mod


_ensure_axon_hooks()

import concourse.bass as bass
import concourse.bacc as bacc
import concourse.tile as tile
import concourse.mybir as mybir
import concourse.bass_utils as bass_utils

N = 8192
NCORES = 8
P = 128
SLOTS = 8
W = 1024          # window width (cols); 2 matmuls of 512
RINGW = 4         # ring capacity in windows (4*1024 = 8 psum banks)
KG = 42           # 6 limb-pair groups x 7
DELTA = 0.1
SIGMA = 0.05
FINAL_SCALE = -1.0 / SIGMA           # -20.0

# I16_MODE: pass1 emits i16 = rne((x+dp)/2^16); pass2 reads it as bf16
# (Schraudolph in the 16-bit domain). Halves pass2 bytes; needs DVE 2x.
I16_MODE = False

A_SC = -100.0 * np.log2(np.e) * 2.0**23
ALPHA = np.float32(A_SC / 2.0**15)
A_SC_EFF = float(ALPHA) * 2.0**15
C_CAL = 460000.0
OFF = 127.0 * 2.0**23 - C_CAL

BF16 = mybir.dt.bfloat16
F32 = mybir.dt.float32
I32 = mybir.dt.int32
I16 = mybir.dt.int16
NPBF16 = ml_dtypes.bfloat16

_cache = {}

# Production order: ('A1', s, w) single-window ACT span; ('A2a'/'A2b', s, w)
# first/second window of a 2048 ACT pair-span; ('D', s, w) vector window
# (boundary if w == s-1, else plain extra). Pairs are placed so their two
# windows land at ring phases (0,1)/(1,2)/(2,3) -- never wrapping.
SEQ_F32 = [
    ('A1', 2, 0), ('A2a', 3, 0), ('A2b', 3, 1), ('D', 1, 0),
    ('D', 2, 1), ('A2a', 4, 1), ('A2b', 4, 2), ('D', 4, 0),
    ('A2a', 5, 0), ('A2b', 5, 1), ('D', 3, 2), ('D', 6, 0),
    ('A2a', 5, 2), ('A2b', 5, 3), ('D', 4, 3), ('D', 5, 4),
    ('A2a', 6, 1), ('A2b', 6, 2), ('A2a', 6, 3), ('A2b', 6, 4),
    ('D', 8, 0), ('A2a', 7, 0), ('A2b', 7, 1), ('D', 6, 5),
    ('A2a', 7, 2), ('A2b', 7, 3), ('A2a', 7, 4), ('A2b', 7, 5),
    ('D', 7, 6), ('A2a', 8, 1), ('A2b', 8, 2), ('D', 8, 7),
    ('A2a', 8, 3), ('A2b', 8, 4), ('A2a', 8, 5), ('A2b', 8, 6),
]

# i16 variant: one more extra (s2w0, s8w1) shifts work to the (cheaper)
# vector stream.  a_s = [0,0,2,2,4,4,6,5].
SEQ_I16 = [
    ('D', 2, 0), ('D', 1, 0), ('A2a', 3, 0), ('A2b', 3, 1),
    ('D', 2, 1), ('A2a', 4, 1), ('A2b', 4, 2), ('D', 4, 0),
    ('A2a', 5, 0), ('A2b', 5, 1), ('D', 3, 2), ('D', 6, 0),
    ('A2a', 5, 2), ('A2b', 5, 3), ('D', 4, 3), ('D', 5, 4),
    ('A2a', 6, 1), ('A2b', 6, 2), ('A2a', 6, 3), ('A2b', 6, 4),
    ('D', 8, 0), ('A2a', 7, 0), ('A2b', 7, 1), ('D', 6, 5),
    ('A2a', 7, 2), ('A2b', 7, 3), ('A2a', 7, 4), ('A2b', 7, 5),
    ('D', 7, 6), ('D', 8, 1), ('A1', 8, 2), ('D', 8, 7),
    ('A2a', 8, 3), ('A2b', 8, 4), ('A2a', 8, 5), ('A2b', 8, 6),
]


def _seq():
    return SEQ_I16 if I16_MODE else SEQ_F32


def _layout():
    """Per-slot D-window counts, i32 region bases, pass2 reduce groups.

    Slots with one D window are laid out first so adjacent pairs of
    regions can be summed by a single [P,2,1024] tensor_reduce.
    """
    seq = _seq()
    d_count = [0] * (SLOTS + 1)
    a_spans = [0] * (SLOTS + 1)
    for kind, s, w in seq:
        if kind == 'D':
            d_count[s] += 1
        elif kind in ('A1', 'A2b'):
            a_spans[s] += 1
    i32_base = [0] * (SLOTS + 1)
    acc = 0
    for s in range(1, SLOTS + 1):
        i32_base[s] = acc
        acc += d_count[s] * W
    i32_total = acc
    groups = [(s,) for s in range(1, SLOTS + 1)]
    return d_count, a_spans, i32_base, i32_total, groups


def _build_body(tc, d):
    nc = tc.nc
    seq = _seq()
    d_count, a_spans, i32_base, i32_total, groups = _layout()
    group_of = {}
    for g in groups:
        for s in g:
            group_of[s] = g

    pass1_dt = I16 if I16_MODE else I32
    pass2_dt = BF16 if I16_MODE else F32

    with tc.tile_pool(name="const", bufs=1) as cpool, \
         tc.tile_pool(name="psum", bufs=1, space="PSUM") as ppool:

        bundle_sb = cpool.tile([P, SLOTS * P], BF16, tag="bundle")
        m_sb = cpool.tile([P, W], BF16, tag="mmask")      # mask on excluded
        rhs_sb = cpool.tile([KG, N], BF16, tag="rhs")
        # odd 512-col halves of rhs, compacted, for the h64 row group
        rhs64_sb = cpool.tile([P, N // 2], BF16, tag="rhs64")
        fb_sb = cpool.tile([P, 2 * SLOTS + 1], F32, tag="fbundle")
        dp_sb = fb_sb[:, 0:SLOTS]
        abias_sb = fb_sb[:, SLOTS: 2 * SLOTS]
        bias2_sb = fb_sb[:, 2 * SLOTS: 2 * SLOTS + 1]
        partials = cpool.tile([P, SLOTS, 6], F32, tag="partials")
        penalty = cpool.tile([P, SLOTS], F32, tag="penalty")
        out_sb = cpool.tile([P, SLOTS], F32, tag="outsb")
        i32_t = cpool.tile([P, i32_total], pass1_dt, tag="i32t")
        junk_a = cpool.tile([P, 2 * W], F32, tag="junka")
        junk_d = cpool.tile([P, 2 * W], pass2_dt, tag="junkd")
        warmt = cpool.tile([P, 2], F32, tag="warmt")

        ring = ppool.tile([P, RINGW * W], F32, tag="ring")

        # --- warmup: trigger the exp ACT_TABLE_LOAD before any DMA work
        # lands on the scalar queue.  Input initialized by gpsimd memset
        # (keeps CoreSim's uninitialized-read check happy).
        nc.gpsimd.memset(warmt[:], 0.0)
        nc.scalar.activation(
            warmt[:, 1:2], warmt[:, 0:1],
            mybir.ActivationFunctionType.Exp, scale=1.0,
        )
        nc.gpsimd.memset(partials[:], 0.0)

        # --- input DMAs.  Each DGE queue moves ~60-115GB/s and they run in
        # parallel, so spread across sync + scalar + gpsimd(SWDGE), ship
        # minimum bytes (h64 gets only the odd 512-col halves it reads),
        # and trickle pieces in the order the window schedule needs them.
        # scalar (the critical consumer queue) gets ONE tiny DMA; the rest
        # go to sync + gpsimd(SWDGE) which are otherwise idle.
        rhs_ap = d["rhs"].ap()
        r64_ap = d["rhs64"].ap()
        bnd_ap = d["bundle"].ap()
        nc.scalar.dma_start(fb_sb[:], d["fbundle"].ap())
        nc.sync.dma_start(bundle_sb[0:KG, :], bnd_ap[0:KG, :])
        nc.gpsimd.dma_start(bundle_sb[64:64 + KG, :], bnd_ap[KG: 2 * KG, :])
        nc.sync.dma_start(rhs_sb[0:KG, 0:1024], rhs_ap[:, 0:1024])
        nc.gpsimd.dma_start(rhs64_sb[64:64 + KG, 0:1024], r64_ap[:, 0:1024])
        nc.sync.dma_start(rhs_sb[0:KG, 1024:2048], rhs_ap[:, 1024:2048])
        nc.gpsimd.dma_start(rhs64_sb[64:64 + KG, 1024:2048],
                            r64_ap[:, 1024:2048])
        nc.sync.dma_start(m_sb[:], d["mmask"].ap())
        nc.sync.dma_start(rhs_sb[0:KG, 2048:4096], rhs_ap[:, 2048:4096])
        nc.gpsimd.dma_start(rhs64_sb[64:64 + KG, 2048:4096],
                            r64_ap[:, 2048:4096])
        nc.sync.dma_start(rhs_sb[0:KG, 4096:8192], rhs_ap[:, 4096:8192])
        nc.gpsimd.dma_start(partials[:, :, 5:6], d["lnsrow"].ap())

        # --- pipeline state
        SPLIT = {6, 8}   # slots whose pass2 runs per window (halves the
                         # serial pass2 tail after the late boundary pass1)
        d_seen = [0] * (SLOTS + 1)      # D pass1s emitted per slot
        ready_pass2 = []                # (slot, i32 off, ncols, pcol)
        next_pcol = [0] * (SLOTS + 1)
        next_p2col = [3] * (SLOTS + 1)

        def pdst(s):
            j = next_pcol[s]
            next_pcol[s] += 1
            return partials[:, s - 1, j: j + 1]

        def emit_pass2(item):
            s, off, nco, j = item
            src = i32_t[:, off: off + nco]
            nc.vector.tensor_scalar(
                junk_d[:, 0:nco], src.bitcast(pass2_dt), 0.0, None,
                op0=mybir.AluOpType.add, op1=mybir.AluOpType.add,
                accum_out=partials[:, s - 1, j: j + 1],
            )

        def emit_span(s, ring_off, nco):
            nc.scalar.activation(
                junk_a[:, 0:nco], ring[:, ring_off: ring_off + nco],
                mybir.ActivationFunctionType.Exp,
                bias=abias_sb[:, s - 1: s],
                scale=SC_ACT,
                accum_out=pdst(s),
            )

        def emit_pass1(s, w, ring_off):
            # D windows fill the slot's i32 region in arrival order; the
            # order within the region does not matter for the sum.
            off = i32_base[s] + d_seen[s] * W
            dst = i32_t[:, off: off + W]
            src = ring[:, ring_off: ring_off + W]
            # pass1 frees a ring slot: raise its priority so the scheduler
            # never queues a (slow, ring-independent) pass2 ahead of it.
            with tc.high_priority(offset=50):
                if w == s - 1:
                    nc.vector.scalar_tensor_tensor(
                        dst, src, dp_sb[:, s - 1: s], m_sb[:],
                        op0=mybir.AluOpType.add, op1=mybir.AluOpType.max,
                    )
                else:
                    nc.vector.tensor_scalar(
                        dst, src, dp_sb[:, s - 1: s], 0.0,
                        op0=mybir.AluOpType.add, op1=mybir.AluOpType.max,
                    )
            d_seen[s] += 1
            if s in SPLIT:
                j = next_p2col[s]
                next_p2col[s] += 1
                ready_pass2.append((s, off, W, j))
            elif d_seen[s] == d_count[s]:
                ready_pass2.append(
                    (s, i32_base[s], d_count[s] * W, next_p2col[s]))

        # --- main loop: 2 matmuls per window + chasing consumers
        for k, (kind, s, w) in enumerate(seq):
            for half in range(2):
                roff = (k % 4) * W + half * 512
                if half == 0:
                    base, rsrc = 0, rhs_sb[0:KG, w * W: w * W + 512]
                else:
                    base = 64
                    rsrc = rhs64_sb[64: 64 + KG, w * 512: w * 512 + 512]
                nc.tensor.matmul(
                    ring[:, roff: roff + 512],
                    lhsT=bundle_sb[base: base + KG, bass.ts(s - 1, P)],
                    rhs=rsrc,
                    start=True,
                    stop=True,
                    tile_position=(base, 0),
                )
            if kind == 'A1':
                emit_span(s, (k % 4) * W, W)
            elif kind == 'A2b':
                emit_span(s, ((k - 1) % 4) * W, 2 * W)
            elif kind == 'D':
                emit_pass1(s, w, (k % 4) * W)
            # flush ready pass2s unless the next unit is another D window
            if kind != 'A2a' and (k == len(seq) - 1 or seq[k + 1][0] != 'D'):
                while ready_pass2:
                    emit_pass2(ready_pass2.pop(0))
        while ready_pass2:
            emit_pass2(ready_pass2.pop(0))

        # one reduce over the (zero-padded) [P, 8, 5] partials; column 4
        # carries ln(s_i)/FINAL_SCALE so the final exp yields s_i * decay
        # directly (bias2 folds out the 2.0s per masked boundary element).
        nc.vector.tensor_reduce(
            penalty[:], partials[:],
            axis=mybir.AxisListType.X, op=mybir.AluOpType.add,
        )
        nc.scalar.activation(
            out_sb[:], penalty[:], mybir.ActivationFunctionType.Exp,
            bias=bias2_sb[:, 0:1], scale=FINAL_SCALE,
        )
        nc.sync.dma_start(d["out"].ap(), out_sb[:])


def _build():
    key = ("nc", I16_MODE)
    if key in _cache:
        return _cache[key]
    nc = bacc.Bacc(
        "TRN2",
        target_bir_lowering=False,
        debug=False,
        enable_asserts=False,
    )
    d = {
        "bundle": nc.dram_tensor(
            "bundle", [2 * KG, SLOTS * P], BF16, kind="ExternalInput"
        ),
        "mmask": nc.dram_tensor("mmask", [P, W], BF16, kind="ExternalInput"),
        "rhs": nc.dram_tensor("rhs", [KG, N], BF16, kind="ExternalInput"),
        "rhs64": nc.dram_tensor(
            "rhs64", [KG, N // 2], BF16, kind="ExternalInput"
        ),
        "fbundle": nc.dram_tensor(
            "fbundle", [P, 2 * SLOTS + 1], F32, kind="ExternalInput"
        ),
        "lnsrow": nc.dram_tensor(
            "lnsrow", [P, SLOTS], F32, kind="ExternalInput"
        ),
        "out": nc.dram_tensor("out", [P, SLOTS], F32, kind="ExternalOutput"),
    }
    with tile.TileContext(nc) as tc:
        _build_body(tc, d)
    nc.compile()
    _cache[key] = nc
    return nc


# Schraudolph domain scaling: in i16 mode everything is divided by 2^16.
if I16_MODE:
    DOM = 2.0**16
    MBIG = 2.0**14    # i16 16384; bitcast bf16 = 2.0 per excluded element
else:
    DOM = 1.0
    MBIG = 2.0**30    # bitcast f32 = 2.0 per excluded element
SC_ACT = float(np.float32(-100.0 / A_SC_EFF)) * DOM


def _split3(x64):
    a0 = x64.astype(NPBF16)
    r = x64 - a0.astype(np.float64)
    a1 = r.astype(NPBF16)
    r2 = r - a1.astype(np.float64)
    a2 = r2.astype(NPBF16)
    return a0, a1, a2


def _prepare_inputs(src_points, tgt_points, scores):
    scores = np.asarray(scores, np.float32)
    src = np.asarray(src_points, np.float32)
    tgt = np.asarray(tgt_points, np.float32)

    order = np.argsort(-scores.astype(np.float64), kind="stable")
    s_sorted = scores[order]
    P6 = np.concatenate([src, tgt], axis=1).astype(np.float64)[order]
    sq = np.sum(P6 * P6, axis=1)

    alpha_eff = ALPHA / DOM
    A7 = np.concatenate([(-2.0 * P6).T, np.ones((1, N))], axis=0) * float(
        np.float32(alpha_eff))
    B7 = np.concatenate([P6.T, sq[None, :]], axis=0) * 2.0**15
    A0, A1, A2 = _split3(A7)
    B0, B1, B2 = _split3(B7)
    lhsT_full = np.concatenate([A0, A1, A0, A1, A0, A2], axis=0)  # [42,N]
    rhs42 = np.ascontiguousarray(
        np.concatenate([B0, B1, B2, B0, B1, B0], axis=0))

    dp_full = ((A_SC_EFF * sq + OFF) / DOM).astype(np.float32)
    abias_full = (-100.0 * sq).astype(np.float32)

    # odd 512-col halves, compacted [42, 4096], for the h64 row group
    rhs64 = np.ascontiguousarray(
        rhs42.reshape(KG, N // W, 2, 512)[:, :, 1, :].reshape(KG, N // 2))

    in_maps = []
    for c in range(NCORES):
        gs = 8 * np.arange(SLOTS) + c
        rows = (gs[:, None] * P + np.arange(P)[None, :]).reshape(-1)
        lhsT_c = lhsT_full[:, rows].astype(NPBF16)
        bundle_c = np.ascontiguousarray(
            np.concatenate([lhsT_c, lhsT_c], axis=0))
        dp_c = dp_full[rows].reshape(SLOTS, P).T
        abias_c = abias_full[rows].reshape(SLOTS, P).T
        srow_c = s_sorted[rows].reshape(SLOTS, P).T.astype(np.float64)
        lnsrow_c = (np.log(np.maximum(srow_c, 1e-30)) / FINAL_SCALE
                    ).astype(np.float32)
        f = np.arange(W)[None, :]
        p = np.arange(P)[:, None]
        m_c = (MBIG * (f >= (P * c + p))).astype(NPBF16)
        # n excluded per row (same for every slot) -> decay bias +40*n
        n_excl = (W - P * c - np.arange(P)).astype(np.float64)
        bias2_c = (40.0 * n_excl).astype(np.float32).reshape(P, 1)
        fbundle_c = np.ascontiguousarray(np.concatenate(
            [dp_c, abias_c, bias2_c], axis=1
        ).astype(np.float32))
        in_maps.append({
            "bundle": bundle_c,
            "mmask": np.ascontiguousarray(m_c),
            "rhs": rhs42,
            "rhs64": rhs64,
            "fbundle": fbundle_c,
            "lnsrow": np.ascontiguousarray(lnsrow_c),
        })
    return in_maps, order, s_sorted, P6


def _tie_correction(out_sorted, s_sorted, P6):
    ties = np.flatnonzero(np.diff(s_sorted) == 0.0)
    if ties.size == 0:
        return out_sorted
    out = out_sorted.copy()
    runs = []
    start = ties[0]
    prev = ties[0]
    for t in ties[1:]:
        if t != prev + 1:
            runs.append((start, prev + 1))
            start = t
        prev = t
    runs.append((start, prev + 1))
    for (a, b) in runs:
        idx = np.arange(a, b + 1)
        for ii in range(1, idx.size):
            i = idx[ii]
            js = idx[:ii]
            d2 = np.sum((P6[i] - P6[js]) ** 2, axis=1)
            corr = np.sum(np.exp(-100.0 * d2))
            out[i] = out[i] * np.exp(-FINAL_SCALE * corr)
    return out


LAST_EXEC_TIME_NS = None


def kernel(src_points, tgt_points, scores):
    global LAST_EXEC_TIME_NS
    nc = _build()
    in_maps, order, s_sorted, P6 = _prepare_inputs(
        src_points, tgt_points, scores)
    res = bass_utils.run_bass_kernel_spmd(
        nc, in_maps, core_ids=list(range(NCORES)))
    LAST_EXEC_TIME_NS = res.exec_time_ns

    out_sorted = np.empty((N // P, P), np.float32)
    for c in range(NCORES):
        gs = 8 * np.arange(SLOTS) + c
        out_sorted[gs, :] = np.asarray(res.results[c]["out"]).T  # [8,128]
    out_sorted = out_sorted.reshape(N)
    out_sorted = _tie_correction(out_sorted, s_sorted, P6)

    out = np.empty(N, np.float32)
    out[order] = out_sorted
    return out


# revision 44
# speedup vs baseline: 1.0260x; 1.0260x over previous
"""Correspondence Soft-NMS on 8 Trainium2 NeuronCores (Bass/Tile), v4.

Math: penalty_i = sum_j [s_j > s_i] * exp(-(d2src_ij + d2tgt_ij)/delta^2)
      out_i    = s_i * exp(-penalty_i / sigma)

Design (v4 = rescheduled v3):
  * Host sorts by score desc -> suppressors are the positional prefix (ties
    fixed by a host-side correction).
  * Pairwise dots are PRE-SCALED into the Schraudolph integer domain:
    psum x = A_SC*(sq_j - 2 P_i.P_j), A_SC = -100*log2(e)*2^23. With
    dp_i = A_SC*sq_i + (127*2^23 - C), rne(max(x+dp,0)) bitcast to f32
    IS ~exp(z) (exact 0 for far pairs).
  * bf16 limbs, 6 limb-pair groups (00,11,02,10,01,20) -> K=42, duplicated
    at partitions 0/64 so consecutive matmuls alternate PE row-groups.
  * 36 windows of 1024 cols (2 matmuls each) stream through a 4-window
    PSUM ring. Window order is hand-scheduled so the two consumer streams
    (ACT on scalar, Schraudolph 2-pass on vector) interleave ~2:1 and all
    ACT pair-spans are ring-contiguous (no phase wrap).
  * ACT stream: exp(SC_ACT*x + bias_i) + fused row-sum, 2048-col spans.
  * DVE stream: pass1 i32 = rne(max(x+dp,0)) per window into a dedicated
    per-slot SBUF region (boundary windows via scalar_tensor_tensor with a
    2^30 mask -> bitcast 2.0 per excluded element, fixed by the final exp
    bias); pass2 sums the bitcast region once per slot with accum_out.
  * Startup: warmup exp (table load) issued first on scalar; input DMAs
    split across sync/scalar queues smallest-first so matmul 0 starts as
    early as possible.
"""

import sys
import types

import numpy as np
import ml_dtypes


def _ensure_axon_hooks():
    try:
        import antenv.axon_hooks  # noqa: F401
        return
    except ImportError:
        pass
    try:
        import antenv
    except ImportError:
        return
    mod = types.ModuleType("antenv.axon_hooks")
    mod._hook = None

    def set_axon_ntff_profile_hook(h):
        mod._hook = h

    def get_axon_ntff_profile_hook():
        return mod._hook

    mod.set_axon_ntff_profile_hook = set_axon_ntff_profile_hook
    mod.get_axon_ntff_profile_hook = get_axon_ntff_profile_hook
    sys.modules["antenv.axon_hooks"] = mod
    antenv.axon_hooks = mod


_ensure_axon_hooks()

import concourse.bass as bass
import concourse.bacc as bacc
import concourse.tile as tile
import concourse.mybir as mybir
import concourse.bass_utils as bass_utils

N = 8192
NCORES = 8
P = 128
SLOTS = 8
W = 1024          # window width (cols); 2 matmuls of 512
RINGW = 4         # ring capacity in windows (4*1024 = 8 psum banks)
KG = 42           # 6 limb-pair groups x 7
DELTA = 0.1
SIGMA = 0.05
FINAL_SCALE = -1.0 / SIGMA           # -20.0

# I16_MODE: pass1 emits i16 = rne((x+dp)/2^16); pass2 reads it as bf16
# (Schraudolph in the 16-bit domain). Halves pass2 bytes; needs DVE 2x.
I16_MODE = False

A_SC = -100.0 * np.log2(np.e) * 2.0**23
ALPHA = np.float32(A_SC / 2.0**15)
A_SC_EFF = float(ALPHA) * 2.0**15
C_CAL = 460000.0
OFF = 127.0 * 2.0**23 - C_CAL

BF16 = mybir.dt.bfloat16
F32 = mybir.dt.float32
I32 = mybir.dt.int32
I16 = mybir.dt.int16
NPBF16 = ml_dtypes.bfloat16

_cache = {}

# Production order: ('A1', s, w) single-window ACT span; ('A2a'/'A2b', s, w)
# first/second window of a 2048 ACT pair-span; ('D', s, w) vector window
# (boundary if w == s-1, else plain extra). Pairs are placed so their two
# windows land at ring phases (0,1)/(1,2)/(2,3) -- never wrapping.
SEQ_F32 = [
    ('A1', 2, 0), ('A2a', 3, 0), ('A2b', 3, 1), ('D', 1, 0),
    ('D', 2, 1), ('A2a', 4, 1), ('A2b', 4, 2), ('D', 4, 0),
    ('A2a', 5, 0), ('A2b', 5, 1), ('D', 3, 2), ('D', 6, 0),
    ('A2a', 5, 2), ('A2b', 5, 3), ('D', 4, 3), ('D', 5, 4),
    ('A2a', 6, 1), ('A2b', 6, 2), ('A2a', 6, 3), ('A2b', 6, 4),
    ('D', 8, 0), ('A2a', 7, 0), ('A2b', 7, 1), ('D', 6, 5),
    ('A2a', 7, 2), ('A2b', 7, 3), ('A2a', 7, 4), ('A2b', 7, 5),
    ('D', 7, 6), ('A2a', 8, 1), ('A2b', 8, 2), ('D', 8, 7),
    ('A2a', 8, 3), ('A2b', 8, 4), ('A2a', 8, 5), ('A2b', 8, 6),
]

# i16 variant: one more extra (s2w0, s8w1) shifts work to the (cheaper)
# vector stream.  a_s = [0,0,2,2,4,4,6,5].
SEQ_I16 = [
    ('D', 2, 0), ('D', 1, 0), ('A2a', 3, 0), ('A2b', 3, 1),
    ('D', 2, 1), ('A2a', 4, 1), ('A2b', 4, 2), ('D', 4, 0),
    ('A2a', 5, 0), ('A2b', 5, 1), ('D', 3, 2), ('D', 6, 0),
    ('A2a', 5, 2), ('A2b', 5, 3), ('D', 4, 3), ('D', 5, 4),
    ('A2a', 6, 1), ('A2b', 6, 2), ('A2a', 6, 3), ('A2b', 6, 4),
    ('D', 8, 0), ('A2a', 7, 0), ('A2b', 7, 1), ('D', 6, 5),
    ('A2a', 7, 2), ('A2b', 7, 3), ('A2a', 7, 4), ('A2b', 7, 5),
    ('D', 7, 6), ('D', 8, 1), ('A1', 8, 2), ('D', 8, 7),
    ('A2a', 8, 3), ('A2b', 8, 4), ('A2a', 8, 5), ('A2b', 8, 6),
]


def _seq():
    return SEQ_I16 if I16_MODE else SEQ_F32


def _layout():
    """Per-slot D-window counts, i32 region bases, pass2 reduce groups.

    Slots with one D window are laid out first so adjacent pairs of
    regions can be summed by a single [P,2,1024] tensor_reduce.
    """
    seq = _seq()
    d_count = [0] * (SLOTS + 1)
    a_spans = [0] * (SLOTS + 1)
    for kind, s, w in seq:
        if kind == 'D':
            d_count[s] += 1
        elif kind in ('A1', 'A2b'):
            a_spans[s] += 1
    i32_base = [0] * (SLOTS + 1)
    acc = 0
    for s in range(1, SLOTS + 1):
        i32_base[s] = acc
        acc += d_count[s] * W
    i32_total = acc
    groups = [(s,) for s in range(1, SLOTS + 1)]
    return d_count, a_spans, i32_base, i32_total, groups


def _build_body(tc, d):
    nc = tc.nc
    seq = _seq()
    d_count, a_spans, i32_base, i32_total, groups = _layout()

    pass1_dt = I16 if I16_MODE else I32
    pass2_dt = BF16 if I16_MODE else F32

    with tc.tile_pool(name="const", bufs=1) as cpool, \
         tc.tile_pool(name="psum", bufs=1, space="PSUM") as ppool:

        bundle_sb = cpool.tile([P, SLOTS * P], BF16, tag="bundle")
        m_sb = cpool.tile([P, W], BF16, tag="mmask")      # mask on excluded
        rhs_sb = cpool.tile([KG, N], BF16, tag="rhs")
        # odd 512-col halves of rhs, compacted, for the h64 row group
        rhs64_sb = cpool.tile([P, N // 2], BF16, tag="rhs64")
        fb_sb = cpool.tile([P, 2 * SLOTS + 1], F32, tag="fbundle")
        dp_sb = fb_sb[:, 0:SLOTS]
        abias_sb = fb_sb[:, SLOTS: 2 * SLOTS]
        bias2_sb = fb_sb[:, 2 * SLOTS: 2 * SLOTS + 1]
        partials = cpool.tile([P, SLOTS, 5], F32, tag="partials")
        penalty = cpool.tile([P, SLOTS], F32, tag="penalty")
        out_sb = cpool.tile([P, SLOTS], F32, tag="outsb")
        i32_t = cpool.tile([P, i32_total], pass1_dt, tag="i32t")
        junk_a = cpool.tile([P, 2 * W], F32, tag="junka")
        junk_d = cpool.tile([P, 2 * W], pass2_dt, tag="junkd")
        warmt = cpool.tile([P, 2], F32, tag="warmt")

        ring = ppool.tile([P, RINGW * W], F32, tag="ring")

        # --- warmup: trigger the exp ACT_TABLE_LOAD before any DMA work
        # lands on the scalar queue.  Input initialized by gpsimd memset
        # (keeps CoreSim's uninitialized-read check happy).
        nc.gpsimd.memset(warmt[:], 0.0)
        nc.scalar.activation(
            warmt[:, 1:2], warmt[:, 0:1],
            mybir.ActivationFunctionType.Exp, scale=1.0,
        )
        nc.gpsimd.memset(partials[:], 0.0)

        # --- input DMAs.  Each DGE queue moves ~60-115GB/s and they run in
        # parallel, so spread across sync + scalar + gpsimd(SWDGE), ship
        # minimum bytes (h64 gets only the odd 512-col halves it reads),
        # and trickle pieces in the order the window schedule needs them.
        # scalar (the critical consumer queue) gets ONE tiny DMA; the rest
        # go to sync + gpsimd(SWDGE) which are otherwise idle.
        rhs_ap = d["rhs"].ap()
        r64_ap = d["rhs64"].ap()
        bnd_ap = d["bundle"].ap()
        nc.scalar.dma_start(fb_sb[:], d["fbundle"].ap())
        nc.sync.dma_start(bundle_sb[0:KG, :], bnd_ap[0:KG, :])
        nc.gpsimd.dma_start(bundle_sb[64:64 + KG, :], bnd_ap[KG: 2 * KG, :])
        nc.sync.dma_start(rhs_sb[0:KG, 0:1024], rhs_ap[:, 0:1024])
        nc.gpsimd.dma_start(rhs64_sb[64:64 + KG, 0:1024], r64_ap[:, 0:1024])
        nc.sync.dma_start(m_sb[:], d["mmask"].ap())
        nc.gpsimd.dma_start(rhs64_sb[64:64 + KG, 1024:2048],
                            r64_ap[:, 1024:2048])
        nc.sync.dma_start(rhs_sb[0:KG, 1024:2048], rhs_ap[:, 1024:2048])
        nc.sync.dma_start(rhs_sb[0:KG, 2048:4096], rhs_ap[:, 2048:4096])
        nc.gpsimd.dma_start(rhs64_sb[64:64 + KG, 2048:4096],
                            r64_ap[:, 2048:4096])
        nc.sync.dma_start(rhs_sb[0:KG, 4096:8192], rhs_ap[:, 4096:8192])
        nc.gpsimd.dma_start(partials[:, :, 4:5], d["lnsrow"].ap())

        # --- pipeline state
        SPLIT = set()    # slots whose pass2 would run per window (a split
                         # was tried and measured slower; keep whole-region)
        d_seen = [0] * (SLOTS + 1)      # D pass1s emitted per slot
        ready_pass2 = []                # (slot, i32 off, ncols, pcol)
        next_pcol = [0] * (SLOTS + 1)
        next_p2col = [3] * (SLOTS + 1)

        def pdst(s):
            j = next_pcol[s]
            next_pcol[s] += 1
            return partials[:, s - 1, j: j + 1]

        def emit_pass2(item):
            s, off, nco, j = item
            src = i32_t[:, off: off + nco]
            nc.vector.tensor_scalar(
                junk_d[:, 0:nco], src.bitcast(pass2_dt), 0.0, None,
                op0=mybir.AluOpType.add, op1=mybir.AluOpType.add,
                accum_out=partials[:, s - 1, j: j + 1],
            )

        def emit_span(s, ring_off, nco):
            nc.scalar.activation(
                junk_a[:, 0:nco], ring[:, ring_off: ring_off + nco],
                mybir.ActivationFunctionType.Exp,
                bias=abias_sb[:, s - 1: s],
                scale=SC_ACT,
                accum_out=pdst(s),
            )

        def emit_pass1(s, w, ring_off):
            # D windows fill the slot's i32 region in arrival order; the
            # order within the region does not matter for the sum.
            off = i32_base[s] + d_seen[s] * W
            dst = i32_t[:, off: off + W]
            src = ring[:, ring_off: ring_off + W]
            # pass1 frees a ring slot: raise its priority so the scheduler
            # never queues a (slow, ring-independent) pass2 ahead of it.
            with tc.high_priority(offset=50):
                if w == s - 1:
                    nc.vector.scalar_tensor_tensor(
                        dst, src, dp_sb[:, s - 1: s], m_sb[:],
                        op0=mybir.AluOpType.add, op1=mybir.AluOpType.max,
                    )
                else:
                    nc.vector.tensor_scalar(
                        dst, src, dp_sb[:, s - 1: s], 0.0,
                        op0=mybir.AluOpType.add, op1=mybir.AluOpType.max,
                    )
            d_seen[s] += 1
            if s in SPLIT:
                j = next_p2col[s]
                next_p2col[s] += 1
                ready_pass2.append((s, off, W, j))
            elif d_seen[s] == d_count[s]:
                ready_pass2.append(
                    (s, i32_base[s], d_count[s] * W, next_p2col[s]))

        # --- main loop: 2 matmuls per window + chasing consumers
        for k, (kind, s, w) in enumerate(seq):
            for half in range(2):
                roff = (k % 4) * W + half * 512
                if half == 0:
                    base, rsrc = 0, rhs_sb[0:KG, w * W: w * W + 512]
                else:
                    base = 64
                    rsrc = rhs64_sb[64: 64 + KG, w * 512: w * 512 + 512]
                nc.tensor.matmul(
                    ring[:, roff: roff + 512],
                    lhsT=bundle_sb[base: base + KG, bass.ts(s - 1, P)],
                    rhs=rsrc,
                    start=True,
                    stop=True,
                    tile_position=(base, 0),
                )
            if kind == 'A1':
                emit_span(s, (k % 4) * W, W)
            elif kind == 'A2b':
                emit_span(s, ((k - 1) % 4) * W, 2 * W)
            elif kind == 'D':
                emit_pass1(s, w, (k % 4) * W)
            # flush ready pass2s unless the next unit is another D window
            if kind != 'A2a' and (k == len(seq) - 1 or seq[k + 1][0] != 'D'):
                while ready_pass2:
                    emit_pass2(ready_pass2.pop(0))
        while ready_pass2:
            emit_pass2(ready_pass2.pop(0))

        # one reduce over the (zero-padded) [P, 8, 5] partials; column 4
        # carries ln(s_i)/FINAL_SCALE so the final exp yields s_i * decay
        # directly (bias2 folds out the 2.0s per masked boundary element).
        nc.vector.tensor_reduce(
            penalty[:], partials[:],
            axis=mybir.AxisListType.X, op=mybir.AluOpType.add,
        )
        nc.scalar.activation(
            out_sb[:], penalty[:], mybir.ActivationFunctionType.Exp,
            bias=bias2_sb[:, 0:1], scale=FINAL_SCALE,
        )
        nc.sync.dma_start(d["out"].ap(), out_sb[:])


def _build():
    key = ("nc", I16_MODE)
    if key in _cache:
        return _cache[key]
    nc = bacc.Bacc(
        "TRN2",
        target_bir_lowering=False,
        debug=False,
        enable_asserts=False,
    )
    d = {
        "bundle": nc.dram_tensor(
            "bundle", [2 * KG, SLOTS * P], BF16, kind="ExternalInput"
        ),
        "mmask": nc.dram_tensor("mmask", [P, W], BF16, kind="ExternalInput"),
        "rhs": nc.dram_tensor("rhs", [KG, N], BF16, kind="ExternalInput"),
        "rhs64": nc.dram_tensor(
            "rhs64", [KG, N // 2], BF16, kind="ExternalInput"
        ),
        "fbundle": nc.dram_tensor(
            "fbundle", [P, 2 * SLOTS + 1], F32, kind="ExternalInput"
        ),
        "lnsrow": nc.dram_tensor(
            "lnsrow", [P, SLOTS], F32, kind="ExternalInput"
        ),
        "out": nc.dram_tensor("out", [P, SLOTS], F32, kind="ExternalOutput"),
    }
    with tile.TileContext(nc) as tc:
        _build_body(tc, d)
    nc.compile()
    _cache[key] = nc
    return nc


# Schraudolph domain scaling: in i16 mode everything is divided by 2^16.
if I16_MODE:
    DOM = 2.0**16
    MBIG = 2.0**14    # i16 16384; bitcast bf16 = 2.0 per excluded element
else:
    DOM = 1.0
    MBIG = 2.0**30    # bitcast f32 = 2.0 per excluded element
SC_ACT = float(np.float32(-100.0 / A_SC_EFF)) * DOM


def _split3(x64):
    a0 = x64.astype(NPBF16)
    r = x64 - a0.astype(np.float64)
    a1 = r.astype(NPBF16)
    r2 = r - a1.astype(np.float64)
    a2 = r2.astype(NPBF16)
    return a0, a1, a2


def _prepare_inputs(src_points, tgt_points, scores):
    scores = np.asarray(scores, np.float32)
    src = np.asarray(src_points, np.float32)
    tgt = np.asarray(tgt_points, np.float32)

    order = np.argsort(-scores.astype(np.float64), kind="stable")
    s_sorted = scores[order]
    P6 = np.concatenate([src, tgt], axis=1).astype(np.float64)[order]
    sq = np.sum(P6 * P6, axis=1)

    alpha_eff = ALPHA / DOM
    A7 = np.concatenate([(-2.0 * P6).T, np.ones((1, N))], axis=0) * float(
        np.float32(alpha_eff))
    B7 = np.concatenate([P6.T, sq[None, :]], axis=0) * 2.0**15
    A0, A1, A2 = _split3(A7)
    B0, B1, B2 = _split3(B7)
    lhsT_full = np.concatenate([A0, A1, A0, A1, A0, A2], axis=0)  # [42,N]
    rhs42 = np.ascontiguousarray(
        np.concatenate([B0, B1, B2, B0, B1, B0], axis=0))

    dp_full = ((A_SC_EFF * sq + OFF) / DOM).astype(np.float32)
    abias_full = (-100.0 * sq).astype(np.float32)

    # odd 512-col halves, compacted [42, 4096], for the h64 row group
    rhs64 = np.ascontiguousarray(
        rhs42.reshape(KG, N // W, 2, 512)[:, :, 1, :].reshape(KG, N // 2))

    in_maps = []
    for c in range(NCORES):
        gs = 8 * np.arange(SLOTS) + c
        rows = (gs[:, None] * P + np.arange(P)[None, :]).reshape(-1)
        lhsT_c = lhsT_full[:, rows].astype(NPBF16)
        bundle_c = np.ascontiguousarray(
            np.concatenate([lhsT_c, lhsT_c], axis=0))
        dp_c = dp_full[rows].reshape(SLOTS, P).T
        abias_c = abias_full[rows].reshape(SLOTS, P).T
        srow_c = s_sorted[rows].reshape(SLOTS, P).T.astype(np.float64)
        lnsrow_c = (np.log(np.maximum(srow_c, 1e-30)) / FINAL_SCALE
                    ).astype(np.float32)
        f = np.arange(W)[None, :]
        p = np.arange(P)[:, None]
        m_c = (MBIG * (f >= (P * c + p))).astype(NPBF16)
        # n excluded per row (same for every slot) -> decay bias +40*n
        n_excl = (W - P * c - np.arange(P)).astype(np.float64)
        bias2_c = (40.0 * n_excl).astype(np.float32).reshape(P, 1)
        fbundle_c = np.ascontiguousarray(np.concatenate(
            [dp_c, abias_c, bias2_c], axis=1
        ).astype(np.float32))
        in_maps.append({
            "bundle": bundle_c,
            "mmask": np.ascontiguousarray(m_c),
            "rhs": rhs42,
            "rhs64": rhs64,
            "fbundle": fbundle_c,
            "lnsrow": np.ascontiguousarray(lnsrow_c),
        })
    return in_maps, order, s_sorted, P6


def _tie_correction(out_sorted, s_sorted, P6):
    ties = np.flatnonzero(np.diff(s_sorted) == 0.0)
    if ties.size == 0:
        return out_sorted
    out = out_sorted.copy()
    runs = []
    start = ties[0]
    prev = ties[0]
    for t in ties[1:]:
        if t != prev + 1:
            runs.append((start, prev + 1))
            start = t
        prev = t
    runs.append((start, prev + 1))
    for (a, b) in runs:
        idx = np.arange(a, b + 1)
        for ii in range(1, idx.size):
            i = idx[ii]
            js = idx[:ii]
            d2 = np.sum((P6[i] - P6[js]) ** 2, axis=1)
            corr = np.sum(np.exp(-100.0 * d2))
            out[i] = out[i] * np.exp(-FINAL_SCALE * corr)
    return out


LAST_EXEC_TIME_NS = None


def kernel(src_points, tgt_points, scores):
    global LAST_EXEC_TIME_NS
    nc = _build()
    in_maps, order, s_sorted, P6 = _prepare_inputs(
        src_points, tgt_points, scores)
    res = bass_utils.run_bass_kernel_spmd(
        nc, in_maps, core_ids=list(range(NCORES)))
    LAST_EXEC_TIME_NS = res.exec_time_ns

    out_sorted = np.empty((N // P, P), np.float32)
    for c in range(NCORES):
        gs = 8 * np.arange(SLOTS) + c
        out_sorted[gs, :] = np.asarray(res.results[c]["out"]).T  # [8,128]
    out_sorted = out_sorted.reshape(N)
    out_sorted = _tie_correction(out_sorted, s_sorted, P6)

    out = np.empty(N, np.float32)
    out[order] = out_sorted
    return out


# revision 45
# speedup vs baseline: 1.0468x; 1.0203x over previous
"""Correspondence Soft-NMS on 8 Trainium2 NeuronCores (Bass/Tile), v4.

Math: penalty_i = sum_j [s_j > s_i] * exp(-(d2src_ij + d2tgt_ij)/delta^2)
      out_i    = s_i * exp(-penalty_i / sigma)

Design (v4 = rescheduled v3):
  * Host sorts by score desc -> suppressors are the positional prefix (ties
    fixed by a host-side correction).
  * Pairwise dots are PRE-SCALED into the Schraudolph integer domain:
    psum x = A_SC*(sq_j - 2 P_i.P_j), A_SC = -100*log2(e)*2^23. With
    dp_i = A_SC*sq_i + (127*2^23 - C), rne(max(x+dp,0)) bitcast to f32
    IS ~exp(z) (exact 0 for far pairs).
  * bf16 limbs, 6 limb-pair groups (00,11,02,10,01,20) -> K=42, duplicated
    at partitions 0/64 so consecutive matmuls alternate PE row-groups.
  * 36 windows of 1024 cols (2 matmuls each) stream through a 4-window
    PSUM ring. Window order is hand-scheduled so the two consumer streams
    (ACT on scalar, Schraudolph 2-pass on vector) interleave ~2:1 and all
    ACT pair-spans are ring-contiguous (no phase wrap).
  * ACT stream: exp(SC_ACT*x + bias_i) + fused row-sum, 2048-col spans.
  * DVE stream: pass1 i32 = rne(max(x+dp,0)) per window into a dedicated
    per-slot SBUF region (boundary windows via scalar_tensor_tensor with a
    2^30 mask -> bitcast 2.0 per excluded element, fixed by the final exp
    bias); pass2 sums the bitcast region once per slot with accum_out.
  * Startup: warmup exp (table load) issued first on scalar; input DMAs
    split across sync/scalar queues smallest-first so matmul 0 starts as
    early as possible.
"""

import sys
import types

import numpy as np
import ml_dtypes


def _ensure_axon_hooks():
    try:
        import antenv.axon_hooks  # noqa: F401
        return
    except ImportError:
        pass
    try:
        import antenv
    except ImportError:
        return
    mod = types.ModuleType("antenv.axon_hooks")
    mod._hook = None

    def set_axon_ntff_profile_hook(h):
        mod._hook = h

    def get_axon_ntff_profile_hook():
        return mod._hook

    mod.set_axon_ntff_profile_hook = set_axon_ntff_profile_hook
    mod.get_axon_ntff_profile_hook = get_axon_ntff_profile_hook
    sys.modules["antenv.axon_hooks"] = mod
    antenv.axon_hooks = mod


_ensure_axon_hooks()

import concourse.bass as bass
import concourse.bacc as bacc
import concourse.tile as tile
import concourse.mybir as mybir
import concourse.bass_utils as bass_utils

N = 8192
NCORES = 8
P = 128
SLOTS = 8
W = 1024          # window width (cols); 2 matmuls of 512
RINGW = 4         # ring capacity in windows (4*1024 = 8 psum banks)
KG = 42           # 6 limb-pair groups x 7
DELTA = 0.1
SIGMA = 0.05
FINAL_SCALE = -1.0 / SIGMA           # -20.0

# I16_MODE: pass1 emits i16 = rne((x+dp)/2^16); pass2 reads it as bf16
# (Schraudolph in the 16-bit domain). Halves pass2 bytes; needs DVE 2x.
I16_MODE = False

A_SC = -100.0 * np.log2(np.e) * 2.0**23
ALPHA = np.float32(A_SC / 2.0**15)
A_SC_EFF = float(ALPHA) * 2.0**15
C_CAL = 460000.0
OFF = 127.0 * 2.0**23 - C_CAL

BF16 = mybir.dt.bfloat16
F32 = mybir.dt.float32
I32 = mybir.dt.int32
I16 = mybir.dt.int16
NPBF16 = ml_dtypes.bfloat16

_cache = {}

# Production order: ('A1', s, w) single-window ACT span; ('A2a'/'A2b', s, w)
# first/second window of a 2048 ACT pair-span; ('D', s, w) vector window
# (boundary if w == s-1, else plain extra). Pairs are placed so their two
# windows land at ring phases (0,1)/(1,2)/(2,3) -- never wrapping.
SEQ_F32 = [
    ('A1', 2, 0), ('A2a', 3, 0), ('A2b', 3, 1), ('D', 1, 0),
    ('D', 2, 1), ('A2a', 4, 1), ('A2b', 4, 2), ('D', 4, 0),
    ('A2a', 5, 0), ('A2b', 5, 1), ('D', 3, 2), ('D', 6, 0),
    ('A2a', 5, 2), ('A2b', 5, 3), ('D', 4, 3), ('D', 5, 4),
    ('A2a', 6, 1), ('A2b', 6, 2), ('A2a', 6, 3), ('A2b', 6, 4),
    ('D', 8, 0), ('A2a', 7, 0), ('A2b', 7, 1), ('D', 6, 5),
    ('A2a', 7, 2), ('A2b', 7, 3), ('A2a', 7, 4), ('A2b', 7, 5),
    ('D', 7, 6), ('A2a', 8, 1), ('A2b', 8, 2), ('D', 8, 7),
    ('A2a', 8, 3), ('A2b', 8, 4), ('A2a', 8, 5), ('A2b', 8, 6),
]

# i16 variant: one more extra (s2w0, s8w1) shifts work to the (cheaper)
# vector stream.  a_s = [0,0,2,2,4,4,6,5].
SEQ_I16 = [
    ('D', 2, 0), ('D', 1, 0), ('A2a', 3, 0), ('A2b', 3, 1),
    ('D', 2, 1), ('A2a', 4, 1), ('A2b', 4, 2), ('D', 4, 0),
    ('A2a', 5, 0), ('A2b', 5, 1), ('D', 3, 2), ('D', 6, 0),
    ('A2a', 5, 2), ('A2b', 5, 3), ('D', 4, 3), ('D', 5, 4),
    ('A2a', 6, 1), ('A2b', 6, 2), ('A2a', 6, 3), ('A2b', 6, 4),
    ('D', 8, 0), ('A2a', 7, 0), ('A2b', 7, 1), ('D', 6, 5),
    ('A2a', 7, 2), ('A2b', 7, 3), ('A2a', 7, 4), ('A2b', 7, 5),
    ('D', 7, 6), ('D', 8, 1), ('A1', 8, 2), ('D', 8, 7),
    ('A2a', 8, 3), ('A2b', 8, 4), ('A2a', 8, 5), ('A2b', 8, 6),
]


def _seq():
    return SEQ_I16 if I16_MODE else SEQ_F32


def _layout():
    """Per-slot D-window counts, i32 region bases, pass2 reduce groups.

    Slots with one D window are laid out first so adjacent pairs of
    regions can be summed by a single [P,2,1024] tensor_reduce.
    """
    seq = _seq()
    d_count = [0] * (SLOTS + 1)
    a_spans = [0] * (SLOTS + 1)
    for kind, s, w in seq:
        if kind == 'D':
            d_count[s] += 1
        elif kind in ('A1', 'A2b'):
            a_spans[s] += 1
    i32_base = [0] * (SLOTS + 1)
    acc = 0
    for s in range(1, SLOTS + 1):
        i32_base[s] = acc
        acc += d_count[s] * W
    i32_total = acc
    groups = [(s,) for s in range(1, SLOTS + 1)]
    return d_count, a_spans, i32_base, i32_total, groups


def _build_body(tc, d):
    nc = tc.nc
    seq = _seq()
    d_count, a_spans, i32_base, i32_total, groups = _layout()

    pass1_dt = I16 if I16_MODE else I32
    pass2_dt = BF16 if I16_MODE else F32

    with tc.tile_pool(name="const", bufs=1) as cpool, \
         tc.tile_pool(name="psum", bufs=1, space="PSUM") as ppool:

        bundle_sb = cpool.tile([P, SLOTS * P], BF16, tag="bundle")
        m_sb = cpool.tile([P, W], BF16, tag="mmask")      # mask on excluded
        rhs_sb = cpool.tile([KG, N], BF16, tag="rhs")
        # odd 512-col halves of rhs, compacted, for the h64 row group
        rhs64_sb = cpool.tile([P, N // 2], BF16, tag="rhs64")
        fb_sb = cpool.tile([P, 2 * SLOTS + 1], F32, tag="fbundle")
        dp_sb = fb_sb[:, 0:SLOTS]
        abias_sb = fb_sb[:, SLOTS: 2 * SLOTS]
        bias2_sb = fb_sb[:, 2 * SLOTS: 2 * SLOTS + 1]
        partials = cpool.tile([P, SLOTS, 5], F32, tag="partials")
        penalty = cpool.tile([P, SLOTS], F32, tag="penalty")
        out_sb = cpool.tile([P, SLOTS], F32, tag="outsb")
        i32_t = cpool.tile([P, i32_total], pass1_dt, tag="i32t")
        junk_a = cpool.tile([P, 2 * W], F32, tag="junka")
        junk_d = cpool.tile([P, 2 * W], pass2_dt, tag="junkd")
        warmt = cpool.tile([P, 2], F32, tag="warmt")

        ring = ppool.tile([P, RINGW * W], F32, tag="ring")

        # --- warmup: trigger the exp ACT_TABLE_LOAD before any DMA work
        # lands on the scalar queue.  Input initialized by gpsimd memset
        # (keeps CoreSim's uninitialized-read check happy).
        nc.gpsimd.memset(warmt[:], 0.0)
        nc.scalar.activation(
            warmt[:, 1:2], warmt[:, 0:1],
            mybir.ActivationFunctionType.Exp, scale=1.0,
        )
        nc.gpsimd.memset(partials[:], 0.0)

        # --- input DMAs.  Each DGE queue moves ~60-115GB/s and they run in
        # parallel, so spread across sync + scalar + gpsimd(SWDGE), ship
        # minimum bytes (h64 gets only the odd 512-col halves it reads),
        # and trickle pieces in the order the window schedule needs them.
        # scalar (the critical consumer queue) gets ONE tiny DMA; the rest
        # go to sync + gpsimd(SWDGE) which are otherwise idle.
        rhs_ap = d["rhs"].ap()
        r64_ap = d["rhs64"].ap()
        bnd_ap = d["bundle"].ap()
        nc.scalar.dma_start(fb_sb[:], d["fbundle"].ap())
        nc.sync.dma_start(bundle_sb[0:KG, :], bnd_ap[0:KG, :])
        nc.gpsimd.dma_start(bundle_sb[64:64 + KG, :], bnd_ap[KG: 2 * KG, :])
        nc.sync.dma_start(rhs_sb[0:KG, 0:1024], rhs_ap[:, 0:1024])
        nc.gpsimd.dma_start(rhs64_sb[64:64 + KG, 0:1024], r64_ap[:, 0:1024])
        nc.sync.dma_start(rhs_sb[0:KG, 1024:2048], rhs_ap[:, 1024:2048])
        nc.gpsimd.dma_start(rhs64_sb[64:64 + KG, 1024:2048],
                            r64_ap[:, 1024:2048])
        nc.gpsimd.dma_start(m_sb[:], d["mmask"].ap())
        nc.sync.dma_start(rhs_sb[0:KG, 2048:4096], rhs_ap[:, 2048:4096])
        nc.gpsimd.dma_start(rhs64_sb[64:64 + KG, 2048:4096],
                            r64_ap[:, 2048:4096])
        nc.sync.dma_start(rhs_sb[0:KG, 4096:8192], rhs_ap[:, 4096:8192])
        nc.gpsimd.dma_start(partials[:, :, 4:5], d["lnsrow"].ap())

        # --- pipeline state
        SPLIT = set()    # slots whose pass2 would run per window (a split
                         # was tried and measured slower; keep whole-region)
        d_seen = [0] * (SLOTS + 1)      # D pass1s emitted per slot
        ready_pass2 = []                # (slot, i32 off, ncols, pcol)
        next_pcol = [0] * (SLOTS + 1)
        next_p2col = [3] * (SLOTS + 1)

        def pdst(s):
            j = next_pcol[s]
            next_pcol[s] += 1
            return partials[:, s - 1, j: j + 1]

        def emit_pass2(item):
            s, off, nco, j = item
            src = i32_t[:, off: off + nco]
            nc.vector.tensor_scalar(
                junk_d[:, 0:nco], src.bitcast(pass2_dt), 0.0, None,
                op0=mybir.AluOpType.add, op1=mybir.AluOpType.add,
                accum_out=partials[:, s - 1, j: j + 1],
            )

        def emit_span(s, ring_off, nco):
            nc.scalar.activation(
                junk_a[:, 0:nco], ring[:, ring_off: ring_off + nco],
                mybir.ActivationFunctionType.Exp,
                bias=abias_sb[:, s - 1: s],
                scale=SC_ACT,
                accum_out=pdst(s),
            )

        def emit_pass1(s, w, ring_off):
            # D windows fill the slot's i32 region in arrival order; the
            # order within the region does not matter for the sum.
            off = i32_base[s] + d_seen[s] * W
            dst = i32_t[:, off: off + W]
            src = ring[:, ring_off: ring_off + W]
            # pass1 frees a ring slot: raise its priority so the scheduler
            # never queues a (slow, ring-independent) pass2 ahead of it.
            with tc.high_priority(offset=50):
                if w == s - 1:
                    nc.vector.scalar_tensor_tensor(
                        dst, src, dp_sb[:, s - 1: s], m_sb[:],
                        op0=mybir.AluOpType.add, op1=mybir.AluOpType.max,
                    )
                else:
                    nc.vector.tensor_scalar(
                        dst, src, dp_sb[:, s - 1: s], 0.0,
                        op0=mybir.AluOpType.add, op1=mybir.AluOpType.max,
                    )
            d_seen[s] += 1
            if s in SPLIT:
                j = next_p2col[s]
                next_p2col[s] += 1
                ready_pass2.append((s, off, W, j))
            elif d_seen[s] == d_count[s]:
                ready_pass2.append(
                    (s, i32_base[s], d_count[s] * W, next_p2col[s]))

        # --- main loop: 2 matmuls per window + chasing consumers
        for k, (kind, s, w) in enumerate(seq):
            for half in range(2):
                roff = (k % 4) * W + half * 512
                if half == 0:
                    base, rsrc = 0, rhs_sb[0:KG, w * W: w * W + 512]
                else:
                    base = 64
                    rsrc = rhs64_sb[64: 64 + KG, w * 512: w * 512 + 512]
                nc.tensor.matmul(
                    ring[:, roff: roff + 512],
                    lhsT=bundle_sb[base: base + KG, bass.ts(s - 1, P)],
                    rhs=rsrc,
                    start=True,
                    stop=True,
                    tile_position=(base, 0),
                )
            if kind == 'A1':
                emit_span(s, (k % 4) * W, W)
            elif kind == 'A2b':
                emit_span(s, ((k - 1) % 4) * W, 2 * W)
            elif kind == 'D':
                emit_pass1(s, w, (k % 4) * W)
            # flush ready pass2s unless the next unit is another D window
            if kind != 'A2a' and (k == len(seq) - 1 or seq[k + 1][0] != 'D'):
                while ready_pass2:
                    emit_pass2(ready_pass2.pop(0))
        while ready_pass2:
            emit_pass2(ready_pass2.pop(0))

        # one reduce over the (zero-padded) [P, 8, 5] partials; column 4
        # carries ln(s_i)/FINAL_SCALE so the final exp yields s_i * decay
        # directly (bias2 folds out the 2.0s per masked boundary element).
        nc.vector.tensor_reduce(
            penalty[:], partials[:],
            axis=mybir.AxisListType.X, op=mybir.AluOpType.add,
        )
        nc.scalar.activation(
            out_sb[:], penalty[:], mybir.ActivationFunctionType.Exp,
            bias=bias2_sb[:, 0:1], scale=FINAL_SCALE,
        )
        nc.sync.dma_start(d["out"].ap(), out_sb[:])


def _build():
    key = ("nc", I16_MODE)
    if key in _cache:
        return _cache[key]
    nc = bacc.Bacc(
        "TRN2",
        target_bir_lowering=False,
        debug=False,
        enable_asserts=False,
    )
    d = {
        "bundle": nc.dram_tensor(
            "bundle", [2 * KG, SLOTS * P], BF16, kind="ExternalInput"
        ),
        "mmask": nc.dram_tensor("mmask", [P, W], BF16, kind="ExternalInput"),
        "rhs": nc.dram_tensor("rhs", [KG, N], BF16, kind="ExternalInput"),
        "rhs64": nc.dram_tensor(
            "rhs64", [KG, N // 2], BF16, kind="ExternalInput"
        ),
        "fbundle": nc.dram_tensor(
            "fbundle", [P, 2 * SLOTS + 1], F32, kind="ExternalInput"
        ),
        "lnsrow": nc.dram_tensor(
            "lnsrow", [P, SLOTS], F32, kind="ExternalInput"
        ),
        "out": nc.dram_tensor("out", [P, SLOTS], F32, kind="ExternalOutput"),
    }
    with tile.TileContext(nc) as tc:
        _build_body(tc, d)
    nc.compile()
    _cache[key] = nc
    return nc


# Schraudolph domain scaling: in i16 mode everything is divided by 2^16.
if I16_MODE:
    DOM = 2.0**16
    MBIG = 2.0**14    # i16 16384; bitcast bf16 = 2.0 per excluded element
else:
    DOM = 1.0
    MBIG = 2.0**30    # bitcast f32 = 2.0 per excluded element
SC_ACT = float(np.float32(-100.0 / A_SC_EFF)) * DOM


def _split3(x64):
    a0 = x64.astype(NPBF16)
    r = x64 - a0.astype(np.float64)
    a1 = r.astype(NPBF16)
    r2 = r - a1.astype(np.float64)
    a2 = r2.astype(NPBF16)
    return a0, a1, a2


def _prepare_inputs(src_points, tgt_points, scores):
    scores = np.asarray(scores, np.float32)
    src = np.asarray(src_points, np.float32)
    tgt = np.asarray(tgt_points, np.float32)

    order = np.argsort(-scores.astype(np.float64), kind="stable")
    s_sorted = scores[order]
    P6 = np.concatenate([src, tgt], axis=1).astype(np.float64)[order]
    sq = np.sum(P6 * P6, axis=1)

    alpha_eff = ALPHA / DOM
    A7 = np.concatenate([(-2.0 * P6).T, np.ones((1, N))], axis=0) * float(
        np.float32(alpha_eff))
    B7 = np.concatenate([P6.T, sq[None, :]], axis=0) * 2.0**15
    A0, A1, A2 = _split3(A7)
    B0, B1, B2 = _split3(B7)
    lhsT_full = np.concatenate([A0, A1, A0, A1, A0, A2], axis=0)  # [42,N]
    rhs42 = np.ascontiguousarray(
        np.concatenate([B0, B1, B2, B0, B1, B0], axis=0))

    dp_full = ((A_SC_EFF * sq + OFF) / DOM).astype(np.float32)
    abias_full = (-100.0 * sq).astype(np.float32)

    # odd 512-col halves, compacted [42, 4096], for the h64 row group
    rhs64 = np.ascontiguousarray(
        rhs42.reshape(KG, N // W, 2, 512)[:, :, 1, :].reshape(KG, N // 2))

    in_maps = []
    for c in range(NCORES):
        gs = 8 * np.arange(SLOTS) + c
        rows = (gs[:, None] * P + np.arange(P)[None, :]).reshape(-1)
        lhsT_c = lhsT_full[:, rows].astype(NPBF16)
        bundle_c = np.ascontiguousarray(
            np.concatenate([lhsT_c, lhsT_c], axis=0))
        dp_c = dp_full[rows].reshape(SLOTS, P).T
        abias_c = abias_full[rows].reshape(SLOTS, P).T
        srow_c = s_sorted[rows].reshape(SLOTS, P).T.astype(np.float64)
        lnsrow_c = (np.log(np.maximum(srow_c, 1e-30)) / FINAL_SCALE
                    ).astype(np.float32)
        f = np.arange(W)[None, :]
        p = np.arange(P)[:, None]
        m_c = (MBIG * (f >= (P * c + p))).astype(NPBF16)
        # n excluded per row (same for every slot) -> decay bias +40*n
        n_excl = (W - P * c - np.arange(P)).astype(np.float64)
        bias2_c = (40.0 * n_excl).astype(np.float32).reshape(P, 1)
        fbundle_c = np.ascontiguousarray(np.concatenate(
            [dp_c, abias_c, bias2_c], axis=1
        ).astype(np.float32))
        in_maps.append({
            "bundle": bundle_c,
            "mmask": np.ascontiguousarray(m_c),
            "rhs": rhs42,
            "rhs64": rhs64,
            "fbundle": fbundle_c,
            "lnsrow": np.ascontiguousarray(lnsrow_c),
        })
    return in_maps, order, s_sorted, P6


def _tie_correction(out_sorted, s_sorted, P6):
    ties = np.flatnonzero(np.diff(s_sorted) == 0.0)
    if ties.size == 0:
        return out_sorted
    out = out_sorted.copy()
    runs = []
    start = ties[0]
    prev = ties[0]
    for t in ties[1:]:
        if t != prev + 1:
            runs.append((start, prev + 1))
            start = t
        prev = t
    runs.append((start, prev + 1))
    for (a, b) in runs:
        idx = np.arange(a, b + 1)
        for ii in range(1, idx.size):
            i = idx[ii]
            js = idx[:ii]
            d2 = np.sum((P6[i] - P6[js]) ** 2, axis=1)
            corr = np.sum(np.exp(-100.0 * d2))
            out[i] = out[i] * np.exp(-FINAL_SCALE * corr)
    return out


LAST_EXEC_TIME_NS = None


def kernel(src_points, tgt_points, scores):
    global LAST_EXEC_TIME_NS
    nc = _build()
    in_maps, order, s_sorted, P6 = _prepare_inputs(
        src_points, tgt_points, scores)
    res = bass_utils.run_bass_kernel_spmd(
        nc, in_maps, core_ids=list(range(NCORES)))
    LAST_EXEC_TIME_NS = res.exec_time_ns

    out_sorted = np.empty((N // P, P), np.float32)
    for c in range(NCORES):
        gs = 8 * np.arange(SLOTS) + c
        out_sorted[gs, :] = np.asarray(res.results[c]["out"]).T  # [8,128]
    out_sorted = out_sorted.reshape(N)
    out_sorted = _tie_correction(out_sorted, s_sorted, P6)

    out = np.empty(N, np.float32)
    out[order] = out_sorted
    return out
